# revision 31
# baseline (speedup 1.0000x reference)
"""Trainium2 Bass kernel for the Aligner module (text/mel conv stacks + pairwise L2).

Strategy: data-parallel over batch B=8 -> one sample per NeuronCore, zero
cross-core communication. Per core:
  - embed x via one-hot matmul (one-hot built on host as input marshalling)
  - 3x conv1d (K=3, VALID) per branch as shifted matmuls, bf16 in / f32 psum
  - pairwise distance via a single augmented matmul per 128-row tile:
        d2[i,j] = sum_c mm[c,i]*(-2*tt[c,j]) + nmm[i]*1 + 1*ntt[j]
    (rows 0..63 = channels, rows 64/65 = norm / ones)
  - out = -sqrt(d2) via ACT sqrt + DVE negate, DMA per 2-tile pair.

Scheduling notes: input DMAs are spread across engines so they run in
parallel right after the startup barrier; text-branch matmuls are emitted
between mel tiles so the PE has independent work while conv epilogues run;
conv epilogues are split at the 512/514 boundary so the next layer's first
tile only waits on a tiny boundary chunk.
"""

import numpy as np
import ml_dtypes

import concourse.bass as bass
import concourse.mybir as mybir
import concourse.tile as tile
from concourse import bacc
from concourse.bass_utils import run_bass_kernel_spmd

# Problem constants (hardcoded per harness contract)
B = 8
TT = 256          # text tokens
TM = 1024         # mel frames
V = 256           # vocab
C = 64            # channels
MEL = 80          # mel bins
TTO = TT - 6      # 250
TMO = TM - 6      # 1018

F32 = mybir.dt.float32
BF16 = mybir.dt.bfloat16

# bias pack columns
B_TB1, B_TB2, B_TB3, B_MB1, B_MB2, B_MB3 = range(6)
IDENT = mybir.ActivationFunctionType.Identity


def _build_nc():
    nc = bacc.Bacc()

    oh_p = nc.declare_dram_parameter("oh", [128, 2 * TT], BF16, isOutput=False)
    m_p = nc.declare_dram_parameter("m", [MEL, TM], BF16, isOutput=False)
    embr_p = nc.declare_dram_parameter("embr", [128, 128], BF16, isOutput=False)
    # w64 pack: cols [0:576) = text layers (l*192 + k*64 + co), [576:768) mw2T,
    # [768:960) mw3T; all [cin, k, cout] with cin on partitions.
    w64_p = nc.declare_dram_parameter("w64", [64, 960], BF16, isOutput=False)
    mw1_p = nc.declare_dram_parameter("mw1", [MEL, 192], BF16, isOutput=False)
    bias_p = nc.declare_dram_parameter("bias", [64, 8], F32, isOutput=False)
    # output as 4 contiguous [<=128 text, 509 mel] blocks (b = jt*2 + ic);
    # host reassembles [250, 1018] and transposes back during unshard.
    out_p = nc.declare_dram_parameter("out", [4, 128, 509], F32, isOutput=True)

    with tile.TileContext(nc) as tc:
        with (
            tc.tile_pool(name="singles", bufs=1) as singles,
            tc.tile_pool(name="dist", bufs=4) as dpool,
            tc.tile_pool(name="psA", bufs=3, space="PSUM") as psA,
            tc.tile_pool(name="psN", bufs=1, space="PSUM") as psN,
            tc.tile_pool(name="psD", bufs=3, space="PSUM") as psD,
        ):
            # ---- input DMAs: one ordered HWDGE queue, critical tensors first ----
            oh_t = singles.tile([128, 2 * TT], BF16)
            nc.sync.dma_start(out=oh_t, in_=oh_p[:, :])
            embr_t = singles.tile([128, 128], BF16)
            nc.sync.dma_start(out=embr_t, in_=embr_p[:, :])
            w64_t = singles.tile([64, 960], BF16)
            nc.sync.dma_start(out=w64_t, in_=w64_p[:, :])
            m_t = singles.tile([MEL, TM], BF16)
            nc.sync.dma_start(out=m_t[:, 0:514], in_=m_p[:, 0:514])
            nc.sync.dma_start(out=m_t[:, 514:TM], in_=m_p[:, 514:TM])
            bias_t = singles.tile([64, 8], F32)
            nc.gpsimd.dma_start(out=bias_t, in_=bias_p[:, :])
            mw1_t = singles.tile([MEL, 192], BF16)
            nc.gpsimd.dma_start(out=mw1_t, in_=mw1_p[:, :])

            # ---- ACT table preloads (after DMA issue, before first use) ----
            dummy = singles.tile([1, 2], F32)
            nc.vector.memset(dummy, 0.0)
            dummy2 = singles.tile([1, 2], F32)
            nc.scalar.sqrt(dummy2, dummy)
            nc.scalar.activation(dummy2, dummy, IDENT, bias=0.0)
            nc.scalar.copy(dummy2, dummy)

            # [ones, zeros] / [zeros, ones] column pairs for the [2, N] norm matmuls
            ones2 = singles.tile([64, 2], BF16)
            nc.vector.memset(ones2[:, 0:1], 1.0)
            nc.vector.memset(ones2[:, 1:2], 0.0)
            zo2 = singles.tile([64, 2], BF16)
            nc.vector.memset(zo2[:, 0:1], 0.0)
            nc.vector.memset(zo2[:, 1:2], 1.0)

            # ---- tiles ----
            tt_aug = singles.tile([66, TT], BF16)
            mm_aug = singles.tile([66, TMO], BF16)
            mm1 = singles.tile([64, TM - 2], BF16)
            mm2 = singles.tile([64, TM - 4], BF16)
            t0 = singles.tile([64, TT], BF16)
            t1 = singles.tile([64, 254], BF16)
            t2 = singles.tile([64, 252], BF16)

            def conv3x(ps, wt, wofs, src, n0, nsz):
                for k in range(3):
                    nc.tensor.matmul(
                        ps[:, 0:nsz],
                        wt[:, wofs + k * 64 : wofs + (k + 1) * 64],
                        src[:, n0 + k : n0 + k + nsz],
                        start=(k == 0),
                        stop=(k == 2),
                    )

            def epi_act(dst_ap, ps_ap, bcol, scale=1.0):
                nc.scalar.activation(
                    out=dst_ap, in_=ps_ap, func=IDENT,
                    bias=bias_t[:, bcol : bcol + 1], scale=scale,
                )

            def epi_dve(dst_ap, ps_ap, bcol, s2=None):
                nc.vector.tensor_scalar(
                    out=dst_ap, in0=ps_ap,
                    scalar1=bias_t[:, bcol : bcol + 1],
                    scalar2=s2,
                    op0=mybir.AluOpType.add,
                    **({"op1": mybir.AluOpType.mult} if s2 is not None else {}),
                )

            # ---- embed ----
            ps_e_full = psA.tile([64, 512], F32, tag="convps")
            ps_e = ps_e_full[:, 0:TT]
            nc.tensor.matmul(ps_e, embr_t[:, 0:64], oh_t[:, 0:TT], start=True, stop=False)
            nc.tensor.matmul(
                ps_e, embr_t[:, 64:128], oh_t[:, TT : 2 * TT], start=False, stop=True
            )
            nc.scalar.copy(t0, ps_e)

            # ---- interleaved mel/text conv emission ----
            # mel conv1 tile0
            ps_m10 = psA.tile([64, 512], F32, tag="convps")
            conv3x(ps_m10, mw1_t, 0, m_t, 0, 512)
            epi_act(mm1[:, 0:512], ps_m10[:, 0:512], B_MB1)
            # text conv1
            ps_t1 = psA.tile([64, 512], F32, tag="convps")
            conv3x(ps_t1, w64_t, 0, t0, 0, 254)
            epi_dve(t1, ps_t1[:, 0:254], B_TB1)
            # mel conv1 tile1 (epilogue split: tiny boundary chunk first)
            ps_m11 = psA.tile([64, 512], F32, tag="convps")
            conv3x(ps_m11, mw1_t, 0, m_t, 512, 510)
            nc.vector.tensor_scalar(
                out=mm1[:, 512:514], in0=ps_m11[:, 0:2],
                scalar1=bias_t[:, B_MB1 : B_MB1 + 1], scalar2=None,
                op0=mybir.AluOpType.add,
            )
            epi_dve(mm1[:, 514:1022], ps_m11[:, 2:510], B_MB1)
            # text conv2
            ps_t2 = psA.tile([64, 512], F32, tag="convps")
            conv3x(ps_t2, w64_t, 192, t1, 0, 252)
            epi_dve(t2, ps_t2[:, 0:252], B_TB2)
            # mel conv2 tile0
            ps_m20 = psA.tile([64, 512], F32, tag="convps")
            conv3x(ps_m20, w64_t, 576, mm1, 0, 512)
            epi_act(mm2[:, 0:512], ps_m20[:, 0:512], B_MB2)
            # text conv3 -> tt_aug rows 0..63 = conv3 + tb3
            ps_t3 = psA.tile([64, 512], F32, tag="convps")
            conv3x(ps_t3, w64_t, 384, t2, 0, 250)
            epi_dve(tt_aug[0:64, 0:TTO], ps_t3[:, 0:TTO], B_TB3)
            # mel conv2 tile1
            ps_m21 = psA.tile([64, 512], F32, tag="convps")
            conv3x(ps_m21, w64_t, 576, mm1, 512, 508)
            nc.vector.tensor_scalar(
                out=mm2[:, 512:514], in0=ps_m21[:, 0:2],
                scalar1=bias_t[:, B_MB2 : B_MB2 + 1], scalar2=None,
                op0=mybir.AluOpType.add,
            )
            epi_dve(mm2[:, 514:1020], ps_m21[:, 2:508], B_MB2)

            # ---- text norm chain: tt_aug rows 64,65 = (ntt, 1.0) ----
            sq_tt = singles.tile([64, TTO], BF16)
            nc.vector.tensor_mul(sq_tt, tt_aug[0:64, 0:TTO], tt_aug[0:64, 0:TTO])
            ps_ntt = psN.tile([2, 1024], F32, tag="norm")
            nc.tensor.matmul(ps_ntt[:, 0:TTO], ones2, sq_tt, start=True, stop=True)
            # row0: ntt + 0 ; row1: 0 + 1
            nc.scalar.activation(
                out=tt_aug[64:66, 0:TTO], in_=ps_ntt[:, 0:TTO], func=IDENT,
                bias=bias_t[0:2, 7:8], scale=1.0,
            )

            # ---- mel conv3 + norms, split in halves at column 512 ----
            sq_mm = singles.tile([64, TMO], BF16)
            ps_nmm = psN.tile([2, 1024], F32, tag="norm")

            # half A; rows 0..63 = -2 * (conv3 + mb3)
            ps_m30 = psA.tile([64, 512], F32, tag="convps")
            conv3x(ps_m30, w64_t, 768, mm2, 0, 512)
            epi_dve(mm_aug[0:64, 0:512], ps_m30[:, 0:512], B_MB3, s2=-2.0)
            nc.vector.tensor_mul(
                sq_mm[:, 0:512], mm_aug[0:64, 0:512], mm_aug[0:64, 0:512]
            )
            nc.tensor.matmul(ps_nmm[:, 0:512], zo2, sq_mm[:, 0:512], start=True, stop=True)
            # row0: 0*0.25 + 1 ; row1: 4*nmm*0.25 + 0
            nc.scalar.activation(
                out=mm_aug[64:66, 0:512], in_=ps_nmm[:, 0:512], func=IDENT,
                bias=bias_t[0:2, 6:7], scale=0.25,
            )
            # half B
            ps_m31 = psA.tile([64, 512], F32, tag="convps")
            conv3x(ps_m31, w64_t, 768, mm2, 512, 506)
            epi_dve(mm_aug[0:64, 512:TMO], ps_m31[:, 0:506], B_MB3, s2=-2.0)
            nc.vector.tensor_mul(
                sq_mm[:, 512:TMO], mm_aug[0:64, 512:TMO], mm_aug[0:64, 512:TMO]
            )
            nc.tensor.matmul(
                ps_nmm[:, 512:TMO], zo2, sq_mm[:, 512:TMO], start=True, stop=True
            )
            nc.scalar.activation(
                out=mm_aug[64:66, 512:TMO], in_=ps_nmm[:, 512:TMO], func=IDENT,
                bias=bias_t[0:2, 6:7], scale=0.25,
            )

            # ---- distance: tt stationary, mm moving; out is [text, mel] ----
            for jt in range(2):
                j0 = jt * 128
                cnt = min(128, TTO - j0)  # 128, 122
                for ic in range(2):
                    i0 = ic * 509
                    nsz = 509
                    psd = psD.tile([128, 512], F32, tag="psd")
                    nc.tensor.matmul(
                        psd[0:cnt, 0:nsz],
                        tt_aug[:, j0 : j0 + cnt],
                        mm_aug[:, i0 : i0 + nsz],
                        start=True,
                        stop=True,
                    )
                    d_s = dpool.tile([128, 512], F32, tag="d_s")
                    d_n = dpool.tile([128, 512], F32, tag="d_n")
                    nc.scalar.sqrt(d_s[0:cnt, 0:nsz], psd[0:cnt, 0:nsz])
                    nc.vector.tensor_scalar_mul(
                        d_n[0:cnt, 0:nsz], d_s[0:cnt, 0:nsz], -1.0
                    )
                    b = jt * 2 + ic
                    eng = nc.sync if b % 2 == 0 else nc.scalar
                    eng.dma_start(
                        out=out_p[b, 0:cnt, :],
                        in_=d_n[0:cnt, 0:nsz],
                    )

    nc.finalize()
    return nc


_NC_CACHE = {}


def _get_nc():
    if "nc" not in _NC_CACHE:
        _NC_CACHE["nc"] = _build_nc()
    return _NC_CACHE["nc"]


def _prep_in_maps(x, m, emb, tw1, tb1, tw2, tb2, tw3, tb3, mw1, mb1, mw2, mb2, mw3, mb3):
    bf16 = ml_dtypes.bfloat16

    # emb [256, 64] -> [128, 2*64]: embr[p, h*64+c] = emb[h*128+p, c]
    embr = (
        np.ascontiguousarray(
            emb.astype(np.float32).reshape(2, 128, C).transpose(1, 0, 2).reshape(128, 128)
        ).astype(bf16)
    )

    def wT(w):  # [cout, cin, 3] -> [cin, 3*cout] with (k, cout) packing
        return np.ascontiguousarray(
            w.astype(np.float32).transpose(1, 2, 0).reshape(w.shape[1], 3 * w.shape[0])
        )

    w64 = np.concatenate(
        [wT(tw1), wT(tw2), wT(tw3), wT(mw2), wT(mw3)], axis=1
    ).astype(bf16)  # [64, 960]
    mw1T = wT(mw1).astype(bf16)  # [80, 192]

    biases = np.zeros((64, 8), np.float32)
    for col, b in zip(range(6), (tb1, tb2, tb3, mb1, mb2, mb3)):
        biases[:, col] = b.astype(np.float32)
    # col 6: bias for mm_aug rows 64,65 -> (0+1, nmm+0); col 7: tt_aug rows -> (ntt+0, 0+1)
    biases[0, 6] = 1.0
    biases[1, 6] = 0.0
    biases[0, 7] = 0.0
    biases[1, 7] = 1.0

    # one-hot encoding of x: oh[v%128, (v//128)*256 + j] = (x[j] == v)
    x_i = np.asarray(x).astype(np.int64)
    m_bf = np.asarray(m).astype(np.float32).astype(bf16)

    in_maps = []
    jj = np.arange(TT)
    for b in range(B):
        oh = np.zeros((128, 2, TT), np.float32)
        xb = x_i[b]
        oh[xb % 128, xb // 128, jj] = 1.0
        in_maps.append(
            {
                "oh": np.ascontiguousarray(oh.reshape(128, 2 * TT)).astype(bf16),
                "m": np.ascontiguousarray(m_bf[b]),
                "embr": embr,
                "w64": w64,
                "mw1": mw1T,
                "bias": biases,
            }
        )
    return in_maps


def _assemble(blocks) -> np.ndarray:
    """[4, 128, 509] device blocks -> [1018, 250] (mel, text)."""
    out2 = np.empty((TTO, TMO), np.float32)
    out2[0:128, 0:509] = blocks[0]
    out2[0:128, 509:TMO] = blocks[1]
    out2[128:TTO, 0:509] = blocks[2][0 : TTO - 128]
    out2[128:TTO, 509:TMO] = blocks[3][0 : TTO - 128]
    return out2.T


def kernel(**inputs) -> np.ndarray:
    nc = _get_nc()
    in_maps = _prep_in_maps(**inputs)
    res = run_bass_kernel_spmd(nc, in_maps, core_ids=list(range(B)))
    out = np.stack([_assemble(np.asarray(res.results[i]["out"])) for i in range(B)])
    return np.ascontiguousarray(out).astype(np.float32)


# revision 36
# speedup vs baseline: 1.0868x; 1.0868x over previous
"""Trainium2 Bass kernel for the Aligner module (text/mel conv stacks + pairwise L2).

Strategy: data-parallel over batch B=8 -> one sample per NeuronCore, zero
cross-core communication. Per core:
  - embed x via one-hot matmul (one-hot built on host as input marshalling)
  - 3x conv1d (K=3, VALID) per branch as shifted matmuls, bf16 in / f32 psum
  - pairwise distance via a single augmented matmul per 128-row tile:
        d2[i,j] = sum_c mm[c,i]*(-2*tt[c,j]) + nmm[i]*1 + 1*ntt[j]
    (rows 0..63 = channels, rows 64/65 = norm / ones)
  - out = -sqrt(d2) via ACT sqrt + DVE negate, DMA per 2-tile pair.

Scheduling notes: input DMAs are spread across engines so they run in
parallel right after the startup barrier; text-branch matmuls are emitted
between mel tiles so the PE has independent work while conv epilogues run;
conv epilogues are split at the 512/514 boundary so the next layer's first
tile only waits on a tiny boundary chunk.
"""

import numpy as np
import ml_dtypes

import concourse.bass as bass
import concourse.mybir as mybir
import concourse.tile as tile
from concourse import bacc
from concourse.bass_utils import run_bass_kernel_spmd

# Problem constants (hardcoded per harness contract)
B = 8
TT = 256          # text tokens
TM = 1024         # mel frames
V = 256           # vocab
C = 64            # channels
MEL = 80          # mel bins
TTO = TT - 6      # 250
TMO = TM - 6      # 1018

F32 = mybir.dt.float32
BF16 = mybir.dt.bfloat16

# bias pack columns
B_TB1, B_TB2, B_TB3, B_MB1, B_MB2, B_MB3 = range(6)
IDENT = mybir.ActivationFunctionType.Identity


def _build_nc():
    nc = bacc.Bacc(num_swdge_queues=2)

    oh_p = nc.declare_dram_parameter("oh", [128, 2 * TT], BF16, isOutput=False)
    m_p = nc.declare_dram_parameter("m", [MEL, TM], BF16, isOutput=False)
    embr_p = nc.declare_dram_parameter("embr", [128, 128], BF16, isOutput=False)
    # w64 pack: cols [0:576) = text layers (l*192 + k*64 + co), [576:768) mw2T,
    # [768:960) mw3T; all [cin, k, cout] with cin on partitions.
    w64_p = nc.declare_dram_parameter("w64", [64, 960], BF16, isOutput=False)
    mw1_p = nc.declare_dram_parameter("mw1", [MEL, 192], BF16, isOutput=False)
    bias_p = nc.declare_dram_parameter("bias", [64, 8], F32, isOutput=False)
    # output as 4 contiguous [<=128 text, 509 mel] blocks (b = jt*2 + ic) in
    # bf16; host reassembles [250, 1018], upcasts, transposes during unshard.
    out_p = nc.declare_dram_parameter("out", [4, 128, 509], BF16, isOutput=True)

    with tile.TileContext(nc) as tc:
        with (
            tc.tile_pool(name="singles", bufs=1) as singles,
            tc.tile_pool(name="dist", bufs=4) as dpool,
            tc.tile_pool(name="psA", bufs=3, space="PSUM") as psA,
            tc.tile_pool(name="psN", bufs=1, space="PSUM") as psN,
            tc.tile_pool(name="psD", bufs=3, space="PSUM") as psD,
        ):
            # ---- input DMAs: one ordered HWDGE queue, critical tensors first ----
            oh_t = singles.tile([128, 2 * TT], BF16)
            nc.sync.dma_start(out=oh_t, in_=oh_p[:, :])
            embr_t = singles.tile([128, 128], BF16)
            nc.sync.dma_start(out=embr_t, in_=embr_p[:, :])
            w64_t = singles.tile([64, 960], BF16)
            nc.sync.dma_start(out=w64_t, in_=w64_p[:, :])
            m_t = singles.tile([MEL, TM], BF16)
            nc.sync.dma_start(out=m_t[:, 0:514], in_=m_p[:, 0:514])
            nc.sync.dma_start(out=m_t[:, 514:TM], in_=m_p[:, 514:TM])
            bias_t = singles.tile([64, 8], F32)
            nc.gpsimd.dma_start(out=bias_t, in_=bias_p[:, :])
            mw1_t = singles.tile([MEL, 192], BF16)
            nc.gpsimd.dma_start(out=mw1_t, in_=mw1_p[:, :])

            # ---- ACT table preloads (after DMA issue, before first use) ----
            dummy = singles.tile([1, 2], F32)
            nc.vector.memset(dummy, 0.0)
            dummy2 = singles.tile([1, 2], F32)
            nc.scalar.sqrt(dummy2, dummy)
            nc.scalar.activation(dummy2, dummy, IDENT, bias=0.0)
            nc.scalar.copy(dummy2, dummy)

            # [ones, zeros] / [zeros, ones] column pairs for the [2, N] norm matmuls
            ones2 = singles.tile([64, 2], BF16)
            nc.vector.memset(ones2[:, 0:1], 1.0)
            nc.vector.memset(ones2[:, 1:2], 0.0)
            zo2 = singles.tile([64, 2], BF16)
            nc.vector.memset(zo2[:, 0:1], 0.0)
            nc.vector.memset(zo2[:, 1:2], 1.0)

            # ---- tiles ----
            tt_aug = singles.tile([66, TT], BF16)
            mm_aug = singles.tile([66, TMO], BF16)
            mm1 = singles.tile([64, TM - 2], BF16)
            mm2 = singles.tile([64, TM - 4], BF16)
            t0 = singles.tile([64, TT], BF16)
            t1 = singles.tile([64, 254], BF16)
            t2 = singles.tile([64, 252], BF16)

            def conv3x(ps, wt, wofs, src, n0, nsz):
                for k in range(3):
                    nc.tensor.matmul(
                        ps[:, 0:nsz],
                        wt[:, wofs + k * 64 : wofs + (k + 1) * 64],
                        src[:, n0 + k : n0 + k + nsz],
                        start=(k == 0),
                        stop=(k == 2),
                    )

            def epi_act(dst_ap, ps_ap, bcol, scale=1.0):
                nc.scalar.activation(
                    out=dst_ap, in_=ps_ap, func=IDENT,
                    bias=bias_t[:, bcol : bcol + 1], scale=scale,
                )

            def epi_dve(dst_ap, ps_ap, bcol, s2=None):
                nc.vector.tensor_scalar(
                    out=dst_ap, in0=ps_ap,
                    scalar1=bias_t[:, bcol : bcol + 1],
                    scalar2=s2,
                    op0=mybir.AluOpType.add,
                    **({"op1": mybir.AluOpType.mult} if s2 is not None else {}),
                )

            # ---- embed ----
            ps_e_full = psA.tile([64, 512], F32, tag="convps")
            ps_e = ps_e_full[:, 0:TT]
            nc.tensor.matmul(ps_e, embr_t[:, 0:64], oh_t[:, 0:TT], start=True, stop=False)
            nc.tensor.matmul(
                ps_e, embr_t[:, 64:128], oh_t[:, TT : 2 * TT], start=False, stop=True
            )
            nc.scalar.copy(t0, ps_e)

            # ---- interleaved mel/text conv emission ----
            # mel conv1 tile0
            ps_m10 = psA.tile([64, 512], F32, tag="convps")
            conv3x(ps_m10, mw1_t, 0, m_t, 0, 512)
            epi_act(mm1[:, 0:512], ps_m10[:, 0:512], B_MB1)
            # text conv1
            ps_t1 = psA.tile([64, 512], F32, tag="convps")
            conv3x(ps_t1, w64_t, 0, t0, 0, 254)
            epi_dve(t1, ps_t1[:, 0:254], B_TB1)
            # mel conv1 tile1 (epilogue split: tiny boundary chunk first)
            ps_m11 = psA.tile([64, 512], F32, tag="convps")
            conv3x(ps_m11, mw1_t, 0, m_t, 512, 510)
            nc.vector.tensor_scalar(
                out=mm1[:, 512:514], in0=ps_m11[:, 0:2],
                scalar1=bias_t[:, B_MB1 : B_MB1 + 1], scalar2=None,
                op0=mybir.AluOpType.add,
            )
            epi_dve(mm1[:, 514:1022], ps_m11[:, 2:510], B_MB1)
            # text conv2
            ps_t2 = psA.tile([64, 512], F32, tag="convps")
            conv3x(ps_t2, w64_t, 192, t1, 0, 252)
            epi_dve(t2, ps_t2[:, 0:252], B_TB2)
            # mel conv2 tile0
            ps_m20 = psA.tile([64, 512], F32, tag="convps")
            conv3x(ps_m20, w64_t, 576, mm1, 0, 512)
            epi_act(mm2[:, 0:512], ps_m20[:, 0:512], B_MB2)
            # text conv3 -> tt_aug rows 0..63 = conv3 + tb3
            ps_t3 = psA.tile([64, 512], F32, tag="convps")
            conv3x(ps_t3, w64_t, 384, t2, 0, 250)
            epi_dve(tt_aug[0:64, 0:TTO], ps_t3[:, 0:TTO], B_TB3)
            # mel conv2 tile1
            ps_m21 = psA.tile([64, 512], F32, tag="convps")
            conv3x(ps_m21, w64_t, 576, mm1, 512, 508)
            nc.vector.tensor_scalar(
                out=mm2[:, 512:514], in0=ps_m21[:, 0:2],
                scalar1=bias_t[:, B_MB2 : B_MB2 + 1], scalar2=None,
                op0=mybir.AluOpType.add,
            )
            epi_dve(mm2[:, 514:1020], ps_m21[:, 2:508], B_MB2)

            # ---- text norm chain: tt_aug rows 64,65 = (ntt, 1.0) ----
            sq_tt = singles.tile([64, TTO], BF16)
            nc.vector.tensor_mul(sq_tt, tt_aug[0:64, 0:TTO], tt_aug[0:64, 0:TTO])
            ps_ntt = psN.tile([2, 1024], F32, tag="norm")
            nc.tensor.matmul(ps_ntt[:, 0:TTO], ones2, sq_tt, start=True, stop=True)
            # row0: ntt + 0 ; row1: 0 + 1
            nc.scalar.activation(
                out=tt_aug[64:66, 0:TTO], in_=ps_ntt[:, 0:TTO], func=IDENT,
                bias=bias_t[0:2, 7:8], scale=1.0,
            )

            # ---- mel conv3 + norms, split in halves at column 512 ----
            sq_mm = singles.tile([64, TMO], BF16)
            ps_nmm = psN.tile([2, 1024], F32, tag="norm")

            # half A; rows 0..63 = -2 * (conv3 + mb3)
            ps_m30 = psA.tile([64, 512], F32, tag="convps")
            conv3x(ps_m30, w64_t, 768, mm2, 0, 512)
            epi_dve(mm_aug[0:64, 0:512], ps_m30[:, 0:512], B_MB3, s2=-2.0)
            nc.vector.tensor_mul(
                sq_mm[:, 0:512], mm_aug[0:64, 0:512], mm_aug[0:64, 0:512]
            )
            nc.tensor.matmul(ps_nmm[:, 0:512], zo2, sq_mm[:, 0:512], start=True, stop=True)
            # row0: 0*0.25 + 1 ; row1: 4*nmm*0.25 + 0
            nc.scalar.activation(
                out=mm_aug[64:66, 0:512], in_=ps_nmm[:, 0:512], func=IDENT,
                bias=bias_t[0:2, 6:7], scale=0.25,
            )
            # half B
            ps_m31 = psA.tile([64, 512], F32, tag="convps")
            conv3x(ps_m31, w64_t, 768, mm2, 512, 506)
            epi_dve(mm_aug[0:64, 512:TMO], ps_m31[:, 0:506], B_MB3, s2=-2.0)
            nc.vector.tensor_mul(
                sq_mm[:, 512:TMO], mm_aug[0:64, 512:TMO], mm_aug[0:64, 512:TMO]
            )
            nc.tensor.matmul(
                ps_nmm[:, 512:TMO], zo2, sq_mm[:, 512:TMO], start=True, stop=True
            )
            nc.scalar.activation(
                out=mm_aug[64:66, 512:TMO], in_=ps_nmm[:, 512:TMO], func=IDENT,
                bias=bias_t[0:2, 6:7], scale=0.25,
            )

            # ---- distance: tt stationary, mm moving; out is [text, mel] ----
            for jt in range(2):
                j0 = jt * 128
                cnt = min(128, TTO - j0)  # 128, 122
                for ic in range(2):
                    i0 = ic * 509
                    nsz = 509
                    psd = psD.tile([128, 512], F32, tag="psd")
                    nc.tensor.matmul(
                        psd[0:cnt, 0:nsz],
                        tt_aug[:, j0 : j0 + cnt],
                        mm_aug[:, i0 : i0 + nsz],
                        start=True,
                        stop=True,
                    )
                    d_s = dpool.tile([128, 512], F32, tag="d_s")
                    d_n = dpool.tile([128, 512], BF16, tag="d_n")
                    nc.scalar.sqrt(d_s[0:cnt, 0:nsz], psd[0:cnt, 0:nsz])
                    nc.vector.tensor_scalar_mul(
                        d_n[0:cnt, 0:nsz], d_s[0:cnt, 0:nsz], -1.0
                    )
                    b = jt * 2 + ic
                    if b == 0:
                        nc.sync.dma_start(out=out_p[b, 0:cnt, :], in_=d_n[0:cnt, 0:nsz])
                    elif b == 1:
                        nc.scalar.dma_start(out=out_p[b, 0:cnt, :], in_=d_n[0:cnt, 0:nsz])
                    else:
                        nc.gpsimd.dma_start(
                            out=out_p[b, 0:cnt, :], in_=d_n[0:cnt, 0:nsz]
                        )

    nc.finalize()
    return nc


_NC_CACHE = {}


def _get_nc():
    if "nc" not in _NC_CACHE:
        _NC_CACHE["nc"] = _build_nc()
    return _NC_CACHE["nc"]


def _prep_in_maps(x, m, emb, tw1, tb1, tw2, tb2, tw3, tb3, mw1, mb1, mw2, mb2, mw3, mb3):
    bf16 = ml_dtypes.bfloat16

    # emb [256, 64] -> [128, 2*64]: embr[p, h*64+c] = emb[h*128+p, c]
    embr = (
        np.ascontiguousarray(
            emb.astype(np.float32).reshape(2, 128, C).transpose(1, 0, 2).reshape(128, 128)
        ).astype(bf16)
    )

    def wT(w):  # [cout, cin, 3] -> [cin, 3*cout] with (k, cout) packing
        return np.ascontiguousarray(
            w.astype(np.float32).transpose(1, 2, 0).reshape(w.shape[1], 3 * w.shape[0])
        )

    w64 = np.concatenate(
        [wT(tw1), wT(tw2), wT(tw3), wT(mw2), wT(mw3)], axis=1
    ).astype(bf16)  # [64, 960]
    mw1T = wT(mw1).astype(bf16)  # [80, 192]

    biases = np.zeros((64, 8), np.float32)
    for col, b in zip(range(6), (tb1, tb2, tb3, mb1, mb2, mb3)):
        biases[:, col] = b.astype(np.float32)
    # col 6: bias for mm_aug rows 64,65 -> (0+1, nmm+0); col 7: tt_aug rows -> (ntt+0, 0+1)
    biases[0, 6] = 1.0
    biases[1, 6] = 0.0
    biases[0, 7] = 0.0
    biases[1, 7] = 1.0

    # one-hot encoding of x: oh[v%128, (v//128)*256 + j] = (x[j] == v)
    x_i = np.asarray(x).astype(np.int64)
    m_bf = np.asarray(m).astype(np.float32).astype(bf16)

    in_maps = []
    jj = np.arange(TT)
    for b in range(B):
        oh = np.zeros((128, 2, TT), np.float32)
        xb = x_i[b]
        oh[xb % 128, xb // 128, jj] = 1.0
        in_maps.append(
            {
                "oh": np.ascontiguousarray(oh.reshape(128, 2 * TT)).astype(bf16),
                "m": np.ascontiguousarray(m_bf[b]),
                "embr": embr,
                "w64": w64,
                "mw1": mw1T,
                "bias": biases,
            }
        )
    return in_maps


def _assemble(blocks) -> np.ndarray:
    """[4, 128, 509] device blocks -> [1018, 250] (mel, text)."""
    blocks = np.asarray(blocks).astype(np.float32)
    out2 = np.empty((TTO, TMO), np.float32)
    out2[0:128, 0:509] = blocks[0]
    out2[0:128, 509:TMO] = blocks[1]
    out2[128:TTO, 0:509] = blocks[2][0 : TTO - 128]
    out2[128:TTO, 509:TMO] = blocks[3][0 : TTO - 128]
    return out2.T


def kernel(**inputs) -> np.ndarray:
    nc = _get_nc()
    in_maps = _prep_in_maps(**inputs)
    res = run_bass_kernel_spmd(nc, in_maps, core_ids=list(range(B)))
    out = np.stack([_assemble(np.asarray(res.results[i]["out"])) for i in range(B)])
    return np.ascontiguousarray(out).astype(np.float32)


# revision 37
# speedup vs baseline: 1.1138x; 1.0248x over previous
"""Trainium2 Bass kernel for the Aligner module (text/mel conv stacks + pairwise L2).

Strategy: data-parallel over batch B=8 -> one sample per NeuronCore, zero
cross-core communication. Per core:
  - embed x via one-hot matmul (one-hot built on host as input marshalling)
  - 3x conv1d (K=3, VALID) per branch as shifted matmuls, bf16 in / f32 psum
  - pairwise distance via a single augmented matmul per 128-row tile:
        d2[i,j] = sum_c mm[c,i]*(-2*tt[c,j]) + nmm[i]*1 + 1*ntt[j]
    (rows 0..63 = channels, rows 64/65 = norm / ones)
  - out = -sqrt(d2) via ACT sqrt + DVE negate, DMA per 2-tile pair.

Scheduling notes: input DMAs are spread across engines so they run in
parallel right after the startup barrier; text-branch matmuls are emitted
between mel tiles so the PE has independent work while conv epilogues run;
conv epilogues are split at the 512/514 boundary so the next layer's first
tile only waits on a tiny boundary chunk.
"""

import numpy as np
import ml_dtypes

import concourse.bass as bass
import concourse.mybir as mybir
import concourse.tile as tile
from concourse import bacc
from concourse.bass_utils import run_bass_kernel_spmd

# Problem constants (hardcoded per harness contract)
B = 8
TT = 256          # text tokens
TM = 1024         # mel frames
V = 256           # vocab
C = 64            # channels
MEL = 80          # mel bins
TTO = TT - 6      # 250
TMO = TM - 6      # 1018

F32 = mybir.dt.float32
BF16 = mybir.dt.bfloat16

# bias pack columns
B_TB1, B_TB2, B_TB3, B_MB1, B_MB2, B_MB3 = range(6)
IDENT = mybir.ActivationFunctionType.Identity


def _build_nc():
    nc = bacc.Bacc(num_swdge_queues=2)

    oh_p = nc.declare_dram_parameter("oh", [128, 2 * TT], BF16, isOutput=False)
    m_p = nc.declare_dram_parameter("m", [MEL, TM], BF16, isOutput=False)
    embr_p = nc.declare_dram_parameter("embr", [128, 128], BF16, isOutput=False)
    # w64 pack: cols [0:576) = text layers (l*192 + k*64 + co), [576:768) mw2T,
    # [768:960) mw3T; all [cin, k, cout] with cin on partitions.
    w64_p = nc.declare_dram_parameter("w64", [64, 960], BF16, isOutput=False)
    mw1_p = nc.declare_dram_parameter("mw1", [MEL, 192], BF16, isOutput=False)
    bias_p = nc.declare_dram_parameter("bias", [64, 8], F32, isOutput=False)
    # output as 4 contiguous [<=128 text, 509 mel] blocks (b = jt*2 + ic) in
    # bf16; host reassembles [250, 1018], upcasts, transposes during unshard.
    out_p = nc.declare_dram_parameter("out", [4, 128, 509], BF16, isOutput=True)

    with tile.TileContext(nc) as tc:
        with (
            tc.tile_pool(name="singles", bufs=1) as singles,
            tc.tile_pool(name="dist", bufs=4) as dpool,
            tc.tile_pool(name="psA", bufs=3, space="PSUM") as psA,
            tc.tile_pool(name="psN", bufs=1, space="PSUM") as psN,
            tc.tile_pool(name="psD", bufs=3, space="PSUM") as psD,
        ):
            # ---- input DMAs: one ordered HWDGE queue, critical tensors first ----
            oh_t = singles.tile([128, 2 * TT], BF16)
            nc.sync.dma_start(out=oh_t, in_=oh_p[:, :])
            embr_t = singles.tile([128, 128], BF16)
            nc.sync.dma_start(out=embr_t, in_=embr_p[:, :])
            w64_t = singles.tile([64, 960], BF16)
            nc.sync.dma_start(out=w64_t, in_=w64_p[:, :])
            m_t = singles.tile([MEL, TM], BF16)
            nc.sync.dma_start(out=m_t[:, 0:514], in_=m_p[:, 0:514])
            nc.sync.dma_start(out=m_t[:, 514:TM], in_=m_p[:, 514:TM])
            bias_t = singles.tile([64, 8], F32)
            nc.gpsimd.dma_start(out=bias_t, in_=bias_p[:, :])
            mw1_t = singles.tile([MEL, 192], BF16)
            nc.gpsimd.dma_start(out=mw1_t, in_=mw1_p[:, :])

            # ---- ACT table preloads (after DMA issue, before first use) ----
            dummy = singles.tile([1, 2], F32)
            nc.vector.memset(dummy, 0.0)
            dummy2 = singles.tile([1, 2], F32)
            nc.scalar.sqrt(dummy2, dummy)
            nc.scalar.activation(dummy2, dummy, IDENT, bias=0.0)
            nc.scalar.copy(dummy2, dummy)

            # [ones, zeros] / [zeros, ones] column pairs for the [2, N] norm matmuls
            ones2 = singles.tile([64, 2], BF16)
            nc.vector.memset(ones2[:, 0:1], 1.0)
            nc.vector.memset(ones2[:, 1:2], 0.0)
            zo2 = singles.tile([64, 2], BF16)
            nc.vector.memset(zo2[:, 0:1], 0.0)
            nc.vector.memset(zo2[:, 1:2], 1.0)

            # ---- tiles ----
            tt_aug = singles.tile([66, TT], BF16)
            mm_aug = singles.tile([66, TMO], BF16)
            mm1 = singles.tile([64, TM - 2], BF16)
            mm2 = singles.tile([64, TM - 4], BF16)
            t0 = singles.tile([64, TT], BF16)
            t1 = singles.tile([64, 254], BF16)
            t2 = singles.tile([64, 252], BF16)

            def conv3x(ps, wt, wofs, src, n0, nsz):
                for k in range(3):
                    nc.tensor.matmul(
                        ps[:, 0:nsz],
                        wt[:, wofs + k * 64 : wofs + (k + 1) * 64],
                        src[:, n0 + k : n0 + k + nsz],
                        start=(k == 0),
                        stop=(k == 2),
                    )

            def epi_act(dst_ap, ps_ap, bcol, scale=1.0):
                nc.scalar.activation(
                    out=dst_ap, in_=ps_ap, func=IDENT,
                    bias=bias_t[:, bcol : bcol + 1], scale=scale,
                )

            def epi_dve(dst_ap, ps_ap, bcol, s2=None):
                nc.vector.tensor_scalar(
                    out=dst_ap, in0=ps_ap,
                    scalar1=bias_t[:, bcol : bcol + 1],
                    scalar2=s2,
                    op0=mybir.AluOpType.add,
                    **({"op1": mybir.AluOpType.mult} if s2 is not None else {}),
                )

            # ---- embed ----
            ps_e_full = psA.tile([64, 512], F32, tag="convps")
            ps_e = ps_e_full[:, 0:TT]
            nc.tensor.matmul(ps_e, embr_t[:, 0:64], oh_t[:, 0:TT], start=True, stop=False)
            nc.tensor.matmul(
                ps_e, embr_t[:, 64:128], oh_t[:, TT : 2 * TT], start=False, stop=True
            )
            nc.scalar.copy(t0, ps_e)

            # ---- interleaved mel/text conv emission ----
            # mel conv1 tile0
            ps_m10 = psA.tile([64, 512], F32, tag="convps")
            conv3x(ps_m10, mw1_t, 0, m_t, 0, 512)
            epi_act(mm1[:, 0:512], ps_m10[:, 0:512], B_MB1)
            # text conv1
            ps_t1 = psA.tile([64, 512], F32, tag="convps")
            conv3x(ps_t1, w64_t, 0, t0, 0, 254)
            epi_dve(t1, ps_t1[:, 0:254], B_TB1)
            # mel conv1 tile1 (epilogue split: tiny boundary chunk first)
            ps_m11 = psA.tile([64, 512], F32, tag="convps")
            conv3x(ps_m11, mw1_t, 0, m_t, 512, 510)
            nc.vector.tensor_scalar(
                out=mm1[:, 512:514], in0=ps_m11[:, 0:2],
                scalar1=bias_t[:, B_MB1 : B_MB1 + 1], scalar2=None,
                op0=mybir.AluOpType.add,
            )
            epi_dve(mm1[:, 514:1022], ps_m11[:, 2:510], B_MB1)
            # text conv2
            ps_t2 = psA.tile([64, 512], F32, tag="convps")
            conv3x(ps_t2, w64_t, 192, t1, 0, 252)
            epi_dve(t2, ps_t2[:, 0:252], B_TB2)
            # mel conv2 tile0
            ps_m20 = psA.tile([64, 512], F32, tag="convps")
            conv3x(ps_m20, w64_t, 576, mm1, 0, 512)
            epi_act(mm2[:, 0:512], ps_m20[:, 0:512], B_MB2)
            # text conv3 -> tt_aug rows 0..63 = conv3 + tb3
            ps_t3 = psA.tile([64, 512], F32, tag="convps")
            conv3x(ps_t3, w64_t, 384, t2, 0, 250)
            epi_dve(tt_aug[0:64, 0:TTO], ps_t3[:, 0:TTO], B_TB3)
            # mel conv2 tile1
            ps_m21 = psA.tile([64, 512], F32, tag="convps")
            conv3x(ps_m21, w64_t, 576, mm1, 512, 508)
            nc.vector.tensor_scalar(
                out=mm2[:, 512:514], in0=ps_m21[:, 0:2],
                scalar1=bias_t[:, B_MB2 : B_MB2 + 1], scalar2=None,
                op0=mybir.AluOpType.add,
            )
            epi_dve(mm2[:, 514:1020], ps_m21[:, 2:508], B_MB2)

            # ---- text norm chain: tt_aug rows 64,65 = (ntt, 1.0) ----
            sq_tt = singles.tile([64, TTO], BF16)
            nc.vector.tensor_mul(sq_tt, tt_aug[0:64, 0:TTO], tt_aug[0:64, 0:TTO])
            ps_ntt = psN.tile([2, 1024], F32, tag="norm")
            nc.tensor.matmul(ps_ntt[:, 0:TTO], ones2, sq_tt, start=True, stop=True)
            # row0: ntt + 0 ; row1: 0 + 1
            nc.scalar.activation(
                out=tt_aug[64:66, 0:TTO], in_=ps_ntt[:, 0:TTO], func=IDENT,
                bias=bias_t[0:2, 7:8], scale=1.0,
            )

            # ---- mel conv3 + norms, split in halves at column 512 ----
            sq_mm = singles.tile([64, TMO], BF16)
            ps_nmm = psN.tile([2, 1024], F32, tag="norm")

            # half A; rows 0..63 = -2 * (conv3 + mb3)
            ps_m30 = psA.tile([64, 512], F32, tag="convps")
            conv3x(ps_m30, w64_t, 768, mm2, 0, 512)
            epi_dve(mm_aug[0:64, 0:512], ps_m30[:, 0:512], B_MB3, s2=-2.0)
            nc.vector.tensor_mul(
                sq_mm[:, 0:512], mm_aug[0:64, 0:512], mm_aug[0:64, 0:512]
            )
            nc.tensor.matmul(ps_nmm[:, 0:512], zo2, sq_mm[:, 0:512], start=True, stop=True)
            # row0: 0*0.25 + 1 ; row1: 4*nmm*0.25 + 0
            nc.scalar.activation(
                out=mm_aug[64:66, 0:512], in_=ps_nmm[:, 0:512], func=IDENT,
                bias=bias_t[0:2, 6:7], scale=0.25,
            )
            # half B
            ps_m31 = psA.tile([64, 512], F32, tag="convps")
            conv3x(ps_m31, w64_t, 768, mm2, 512, 506)
            epi_dve(mm_aug[0:64, 512:TMO], ps_m31[:, 0:506], B_MB3, s2=-2.0)
            nc.vector.tensor_mul(
                sq_mm[:, 512:TMO], mm_aug[0:64, 512:TMO], mm_aug[0:64, 512:TMO]
            )
            nc.tensor.matmul(
                ps_nmm[:, 512:TMO], zo2, sq_mm[:, 512:TMO], start=True, stop=True
            )
            nc.scalar.activation(
                out=mm_aug[64:66, 512:TMO], in_=ps_nmm[:, 512:TMO], func=IDENT,
                bias=bias_t[0:2, 6:7], scale=0.25,
            )

            # ---- distance: tt stationary, mm moving; out is [text, mel] ----
            for jt in range(2):
                j0 = jt * 128
                cnt = min(128, TTO - j0)  # 128, 122
                for ic in range(2):
                    i0 = ic * 509
                    nsz = 509
                    psd = psD.tile([128, 512], F32, tag="psd")
                    nc.tensor.matmul(
                        psd[0:cnt, 0:nsz],
                        tt_aug[:, j0 : j0 + cnt],
                        mm_aug[:, i0 : i0 + nsz],
                        start=True,
                        stop=True,
                    )
                    d_s = dpool.tile([128, 512], F32, tag="d_s")
                    d_n = dpool.tile([128, 512], BF16, tag="d_n")
                    nc.scalar.sqrt(d_s[0:cnt, 0:nsz], psd[0:cnt, 0:nsz])
                    nc.vector.tensor_scalar_mul(
                        d_n[0:cnt, 0:nsz], d_s[0:cnt, 0:nsz], -1.0
                    )
                    b = jt * 2 + ic
                    eng = nc.sync if b % 2 == 0 else nc.scalar
                    eng.dma_start(out=out_p[b, 0:cnt, :], in_=d_n[0:cnt, 0:nsz])

    nc.finalize()
    return nc


_NC_CACHE = {}


def _get_nc():
    if "nc" not in _NC_CACHE:
        _NC_CACHE["nc"] = _build_nc()
    return _NC_CACHE["nc"]


def _prep_in_maps(x, m, emb, tw1, tb1, tw2, tb2, tw3, tb3, mw1, mb1, mw2, mb2, mw3, mb3):
    bf16 = ml_dtypes.bfloat16

    # emb [256, 64] -> [128, 2*64]: embr[p, h*64+c] = emb[h*128+p, c]
    embr = (
        np.ascontiguousarray(
            emb.astype(np.float32).reshape(2, 128, C).transpose(1, 0, 2).reshape(128, 128)
        ).astype(bf16)
    )

    def wT(w):  # [cout, cin, 3] -> [cin, 3*cout] with (k, cout) packing
        return np.ascontiguousarray(
            w.astype(np.float32).transpose(1, 2, 0).reshape(w.shape[1], 3 * w.shape[0])
        )

    w64 = np.concatenate(
        [wT(tw1), wT(tw2), wT(tw3), wT(mw2), wT(mw3)], axis=1
    ).astype(bf16)  # [64, 960]
    mw1T = wT(mw1).astype(bf16)  # [80, 192]

    biases = np.zeros((64, 8), np.float32)
    for col, b in zip(range(6), (tb1, tb2, tb3, mb1, mb2, mb3)):
        biases[:, col] = b.astype(np.float32)
    # col 6: bias for mm_aug rows 64,65 -> (0+1, nmm+0); col 7: tt_aug rows -> (ntt+0, 0+1)
    biases[0, 6] = 1.0
    biases[1, 6] = 0.0
    biases[0, 7] = 0.0
    biases[1, 7] = 1.0

    # one-hot encoding of x: oh[v%128, (v//128)*256 + j] = (x[j] == v)
    x_i = np.asarray(x).astype(np.int64)
    m_bf = np.asarray(m).astype(np.float32).astype(bf16)

    in_maps = []
    jj = np.arange(TT)
    for b in range(B):
        oh = np.zeros((128, 2, TT), np.float32)
        xb = x_i[b]
        oh[xb % 128, xb // 128, jj] = 1.0
        in_maps.append(
            {
                "oh": np.ascontiguousarray(oh.reshape(128, 2 * TT)).astype(bf16),
                "m": np.ascontiguousarray(m_bf[b]),
                "embr": embr,
                "w64": w64,
                "mw1": mw1T,
                "bias": biases,
            }
        )
    return in_maps


def _assemble(blocks) -> np.ndarray:
    """[4, 128, 509] device blocks -> [1018, 250] (mel, text)."""
    blocks = np.asarray(blocks).astype(np.float32)
    out2 = np.empty((TTO, TMO), np.float32)
    out2[0:128, 0:509] = blocks[0]
    out2[0:128, 509:TMO] = blocks[1]
    out2[128:TTO, 0:509] = blocks[2][0 : TTO - 128]
    out2[128:TTO, 509:TMO] = blocks[3][0 : TTO - 128]
    return out2.T


def kernel(**inputs) -> np.ndarray:
    nc = _get_nc()
    in_maps = _prep_in_maps(**inputs)
    res = run_bass_kernel_spmd(nc, in_maps, core_ids=list(range(B)))
    out = np.stack([_assemble(np.asarray(res.results[i]["out"])) for i in range(B)])
    return np.ascontiguousarray(out).astype(np.float32)


# revision 41
# speedup vs baseline: 1.3883x; 1.2465x over previous
"""Trainium2 Bass kernel for the Aligner module (text/mel conv stacks + pairwise L2).

Strategy: data-parallel over batch B=8 -> one sample per NeuronCore, zero
cross-core communication. Per core:
  - embed x via one-hot matmul (one-hot built on host as input marshalling)
  - 3x conv1d (K=3, VALID) per branch as shifted matmuls, bf16 in / f32 psum
  - pairwise distance via a single augmented matmul per 128-row tile:
        d2[i,j] = sum_c mm[c,i]*(-2*tt[c,j]) + nmm[i]*1 + 1*ntt[j]
    (rows 0..63 = channels, rows 64/65 = norm / ones)
  - out = -sqrt(d2) via ACT sqrt + DVE negate, DMA per 2-tile pair.

Scheduling notes: input DMAs are spread across engines so they run in
parallel right after the startup barrier; text-branch matmuls are emitted
between mel tiles so the PE has independent work while conv epilogues run;
conv epilogues are split at the 512/514 boundary so the next layer's first
tile only waits on a tiny boundary chunk.
"""

import numpy as np
import ml_dtypes

import concourse.bass as bass
import concourse.mybir as mybir
import concourse.tile as tile
from concourse import bacc
from concourse.bass_utils import run_bass_kernel_spmd

# Problem constants (hardcoded per harness contract)
B = 8
TT = 256          # text tokens
TM = 1024         # mel frames
V = 256           # vocab
C = 64            # channels
MEL = 80          # mel bins
TTO = TT - 6      # 250
TMO = TM - 6      # 1018

F32 = mybir.dt.float32
BF16 = mybir.dt.bfloat16

# bias pack columns
B_TB1, B_TB2, B_TB3, B_MB1, B_MB2, B_MB3 = range(6)
IDENT = mybir.ActivationFunctionType.Identity


def _build_nc():
    nc = bacc.Bacc(num_swdge_queues=2)

    oh_p = nc.declare_dram_parameter("oh", [128, 2 * TT], BF16, isOutput=False)
    m_p = nc.declare_dram_parameter("m", [MEL, TM], BF16, isOutput=False)
    embr_p = nc.declare_dram_parameter("embr", [128, 128], BF16, isOutput=False)
    # w64 pack: cols [0:576) = text layers (l*192 + k*64 + co), [576:768) mw2T,
    # [768:960) mw3T; all [cin, k, cout] with cin on partitions.
    w64_p = nc.declare_dram_parameter("w64", [64, 960], BF16, isOutput=False)
    mw1_p = nc.declare_dram_parameter("mw1", [MEL, 192], BF16, isOutput=False)
    bias_p = nc.declare_dram_parameter("bias", [64, 8], F32, isOutput=False)
    # output as 4 contiguous [<=128 text, 509 mel] blocks (b = jt*2 + ic) in
    # bf16; host reassembles [250, 1018], upcasts, transposes during unshard.
    out_p = nc.declare_dram_parameter("out", [4, 128, 509], BF16, isOutput=True)

    with tile.TileContext(nc) as tc:
        with (
            tc.tile_pool(name="singles", bufs=1) as singles,
            tc.tile_pool(name="dist", bufs=4) as dpool,
            tc.tile_pool(name="psA", bufs=3, space="PSUM") as psA,
            tc.tile_pool(name="psN", bufs=1, space="PSUM") as psN,
            tc.tile_pool(name="psD", bufs=3, space="PSUM") as psD,
        ):
            # ---- input DMAs: one ordered HWDGE queue, critical tensors first ----
            embr_t = singles.tile([128, 128], BF16)
            nc.sync.dma_start(out=embr_t, in_=embr_p[:, :])
            oh_t = singles.tile([128, 2 * TT], BF16)
            nc.sync.dma_start(out=oh_t[:, 0:TT], in_=oh_p[:, 0:TT])
            nc.sync.dma_start(out=oh_t[:, TT : 2 * TT], in_=oh_p[:, TT : 2 * TT])
            w64_t = singles.tile([64, 960], BF16)
            nc.sync.dma_start(out=w64_t, in_=w64_p[:, :])
            m_t = singles.tile([MEL, TM], BF16)
            nc.sync.dma_start(out=m_t[:, 0:514], in_=m_p[:, 0:514])
            nc.sync.dma_start(out=m_t[:, 514:TM], in_=m_p[:, 514:TM])
            bias_t = singles.tile([64, 8], F32)
            nc.gpsimd.dma_start(out=bias_t, in_=bias_p[:, :])
            mw1_t = singles.tile([MEL, 192], BF16)
            nc.gpsimd.dma_start(out=mw1_t, in_=mw1_p[:, :])

            # ---- ACT table preloads (after DMA issue, before first use) ----
            dummy = singles.tile([1, 2], F32)
            nc.vector.memset(dummy, 0.0)
            dummy2 = singles.tile([1, 2], F32)
            nc.scalar.sqrt(dummy2, dummy)
            nc.scalar.activation(dummy2, dummy, IDENT, bias=0.0)
            nc.scalar.copy(dummy2, dummy)

            # [ones, zeros] / [zeros, ones] column pairs for the [2, N] norm matmuls
            ones2 = singles.tile([64, 2], BF16)
            nc.vector.memset(ones2[:, 0:1], 1.0)
            nc.vector.memset(ones2[:, 1:2], 0.0)
            zo2 = singles.tile([64, 2], BF16)
            nc.vector.memset(zo2[:, 0:1], 0.0)
            nc.vector.memset(zo2[:, 1:2], 1.0)

            # ---- tiles ----
            # all 4 distance output blocks live in one tile so a single
            # dma_start ships them (one instruction spreads its packets
            # across all 16 DMA engines; later stores on a queue do not)
            d_all = singles.tile([128, 2048], BF16)
            nc.vector.memset(d_all, 0.0)
            tt_aug = singles.tile([66, TT], BF16)
            mm_aug = singles.tile([66, TMO], BF16)
            mm1 = singles.tile([64, TM - 2], BF16)
            mm2 = singles.tile([64, TM - 4], BF16)
            t0 = singles.tile([64, TT], BF16)
            t1 = singles.tile([64, 254], BF16)
            t2 = singles.tile([64, 252], BF16)

            def conv3x(ps, wt, wofs, src, n0, nsz):
                for k in range(3):
                    nc.tensor.matmul(
                        ps[:, 0:nsz],
                        wt[:, wofs + k * 64 : wofs + (k + 1) * 64],
                        src[:, n0 + k : n0 + k + nsz],
                        start=(k == 0),
                        stop=(k == 2),
                    )

            def epi_act(dst_ap, ps_ap, bcol, scale=1.0):
                nc.scalar.activation(
                    out=dst_ap, in_=ps_ap, func=IDENT,
                    bias=bias_t[:, bcol : bcol + 1], scale=scale,
                )

            def epi_dve(dst_ap, ps_ap, bcol, s2=None):
                nc.vector.tensor_scalar(
                    out=dst_ap, in0=ps_ap,
                    scalar1=bias_t[:, bcol : bcol + 1],
                    scalar2=s2,
                    op0=mybir.AluOpType.add,
                    **({"op1": mybir.AluOpType.mult} if s2 is not None else {}),
                )

            # ---- embed ----
            ps_e_full = psA.tile([64, 512], F32, tag="convps")
            ps_e = ps_e_full[:, 0:TT]
            nc.tensor.matmul(ps_e, embr_t[:, 0:64], oh_t[:, 0:TT], start=True, stop=False)
            nc.tensor.matmul(
                ps_e, embr_t[:, 64:128], oh_t[:, TT : 2 * TT], start=False, stop=True
            )
            nc.scalar.copy(t0, ps_e)

            # ---- interleaved mel/text conv emission ----
            # mel conv1 tile0
            ps_m10 = psA.tile([64, 512], F32, tag="convps")
            conv3x(ps_m10, mw1_t, 0, m_t, 0, 512)
            epi_act(mm1[:, 0:512], ps_m10[:, 0:512], B_MB1)
            # text conv1
            ps_t1 = psA.tile([64, 512], F32, tag="convps")
            conv3x(ps_t1, w64_t, 0, t0, 0, 254)
            epi_dve(t1, ps_t1[:, 0:254], B_TB1)
            # mel conv1 tile1 (epilogue split: tiny boundary chunk first)
            ps_m11 = psA.tile([64, 512], F32, tag="convps")
            conv3x(ps_m11, mw1_t, 0, m_t, 512, 510)
            nc.vector.tensor_scalar(
                out=mm1[:, 512:514], in0=ps_m11[:, 0:2],
                scalar1=bias_t[:, B_MB1 : B_MB1 + 1], scalar2=None,
                op0=mybir.AluOpType.add,
            )
            epi_dve(mm1[:, 514:1022], ps_m11[:, 2:510], B_MB1)
            # text conv2
            ps_t2 = psA.tile([64, 512], F32, tag="convps")
            conv3x(ps_t2, w64_t, 192, t1, 0, 252)
            epi_dve(t2, ps_t2[:, 0:252], B_TB2)
            # mel conv2 tile0
            ps_m20 = psA.tile([64, 512], F32, tag="convps")
            conv3x(ps_m20, w64_t, 576, mm1, 0, 512)
            epi_act(mm2[:, 0:512], ps_m20[:, 0:512], B_MB2)
            # text conv3 -> tt_aug rows 0..63 = conv3 + tb3
            ps_t3 = psA.tile([64, 512], F32, tag="convps")
            conv3x(ps_t3, w64_t, 384, t2, 0, 250)
            epi_dve(tt_aug[0:64, 0:TTO], ps_t3[:, 0:TTO], B_TB3)
            # mel conv2 tile1
            ps_m21 = psA.tile([64, 512], F32, tag="convps")
            conv3x(ps_m21, w64_t, 576, mm1, 512, 508)
            nc.vector.tensor_scalar(
                out=mm2[:, 512:514], in0=ps_m21[:, 0:2],
                scalar1=bias_t[:, B_MB2 : B_MB2 + 1], scalar2=None,
                op0=mybir.AluOpType.add,
            )
            epi_dve(mm2[:, 514:1020], ps_m21[:, 2:508], B_MB2)

            # ---- text norm chain: tt_aug rows 64,65 = (ntt, 1.0) ----
            sq_tt = singles.tile([64, TTO], BF16)
            nc.vector.tensor_mul(sq_tt, tt_aug[0:64, 0:TTO], tt_aug[0:64, 0:TTO])
            ps_ntt = psN.tile([2, 1024], F32, tag="norm")
            nc.tensor.matmul(ps_ntt[:, 0:TTO], ones2, sq_tt, start=True, stop=True)
            # row0: ntt + 0 ; row1: 0 + 1
            nc.scalar.activation(
                out=tt_aug[64:66, 0:TTO], in_=ps_ntt[:, 0:TTO], func=IDENT,
                bias=bias_t[0:2, 7:8], scale=1.0,
            )

            # ---- mel conv3 + norms, split in halves at column 512 ----
            sq_mm = singles.tile([64, TMO], BF16)
            ps_nmm = psN.tile([2, 1024], F32, tag="norm")

            # half A; rows 0..63 = -2 * (conv3 + mb3)
            ps_m30 = psA.tile([64, 512], F32, tag="convps")
            conv3x(ps_m30, w64_t, 768, mm2, 0, 512)
            epi_dve(mm_aug[0:64, 0:512], ps_m30[:, 0:512], B_MB3, s2=-2.0)
            nc.vector.tensor_mul(
                sq_mm[:, 0:512], mm_aug[0:64, 0:512], mm_aug[0:64, 0:512]
            )
            nc.tensor.matmul(ps_nmm[:, 0:512], zo2, sq_mm[:, 0:512], start=True, stop=True)
            # row0: 0*0.25 + 1 ; row1: 4*nmm*0.25 + 0
            nc.scalar.activation(
                out=mm_aug[64:66, 0:512], in_=ps_nmm[:, 0:512], func=IDENT,
                bias=bias_t[0:2, 6:7], scale=0.25,
            )
            # half B
            ps_m31 = psA.tile([64, 512], F32, tag="convps")
            conv3x(ps_m31, w64_t, 768, mm2, 512, 506)
            epi_dve(mm_aug[0:64, 512:TMO], ps_m31[:, 0:506], B_MB3, s2=-2.0)
            nc.vector.tensor_mul(
                sq_mm[:, 512:TMO], mm_aug[0:64, 512:TMO], mm_aug[0:64, 512:TMO]
            )
            nc.tensor.matmul(
                ps_nmm[:, 512:TMO], zo2, sq_mm[:, 512:TMO], start=True, stop=True
            )
            nc.scalar.activation(
                out=mm_aug[64:66, 512:TMO], in_=ps_nmm[:, 512:TMO], func=IDENT,
                bias=bias_t[0:2, 6:7], scale=0.25,
            )

            # ---- distance: tt stationary, mm moving; out is [text, mel] ----
            for jt in range(2):
                j0 = jt * 128
                cnt = min(128, TTO - j0)  # 128, 122
                for ic in range(2):
                    i0 = ic * 509
                    nsz = 509
                    psd = psD.tile([128, 512], F32, tag="psd")
                    nc.tensor.matmul(
                        psd[0:cnt, 0:nsz],
                        tt_aug[:, j0 : j0 + cnt],
                        mm_aug[:, i0 : i0 + nsz],
                        start=True,
                        stop=True,
                    )
                    d_s = dpool.tile([128, 512], F32, tag="d_s")
                    nc.scalar.sqrt(d_s[0:cnt, 0:nsz], psd[0:cnt, 0:nsz])
                    b = jt * 2 + ic
                    nc.vector.tensor_scalar_mul(
                        d_all[0:cnt, b * 512 : b * 512 + nsz], d_s[0:cnt, 0:nsz], -1.0
                    )

            # single store instruction for the whole [4, 128, 509] output
            src = bass.AP(
                tensor=d_all.tensor,
                offset=d_all.offset,
                ap=[d_all.ap[0], [512, 4], [1, 509]],
            )
            base = out_p[:, :, :]
            dst = bass.AP(
                tensor=base.tensor,
                offset=base.offset,
                ap=[[509, 128], [128 * 509, 4], [1, 509]],
            )
            nc.scalar.dma_start(out=dst, in_=src)

    nc.finalize()
    return nc


_NC_CACHE = {}


def _get_nc():
    if "nc" not in _NC_CACHE:
        _NC_CACHE["nc"] = _build_nc()
    return _NC_CACHE["nc"]


def _prep_in_maps(x, m, emb, tw1, tb1, tw2, tb2, tw3, tb3, mw1, mb1, mw2, mb2, mw3, mb3):
    bf16 = ml_dtypes.bfloat16

    # emb [256, 64] -> [128, 2*64]: embr[p, h*64+c] = emb[h*128+p, c]
    embr = (
        np.ascontiguousarray(
            emb.astype(np.float32).reshape(2, 128, C).transpose(1, 0, 2).reshape(128, 128)
        ).astype(bf16)
    )

    def wT(w):  # [cout, cin, 3] -> [cin, 3*cout] with (k, cout) packing
        return np.ascontiguousarray(
            w.astype(np.float32).transpose(1, 2, 0).reshape(w.shape[1], 3 * w.shape[0])
        )

    w64 = np.concatenate(
        [wT(tw1), wT(tw2), wT(tw3), wT(mw2), wT(mw3)], axis=1
    ).astype(bf16)  # [64, 960]
    mw1T = wT(mw1).astype(bf16)  # [80, 192]

    biases = np.zeros((64, 8), np.float32)
    for col, b in zip(range(6), (tb1, tb2, tb3, mb1, mb2, mb3)):
        biases[:, col] = b.astype(np.float32)
    # col 6: bias for mm_aug rows 64,65 -> (0+1, nmm+0); col 7: tt_aug rows -> (ntt+0, 0+1)
    biases[0, 6] = 1.0
    biases[1, 6] = 0.0
    biases[0, 7] = 0.0
    biases[1, 7] = 1.0

    # one-hot encoding of x: oh[v%128, (v//128)*256 + j] = (x[j] == v)
    x_i = np.asarray(x).astype(np.int64)
    m_bf = np.asarray(m).astype(np.float32).astype(bf16)

    in_maps = []
    jj = np.arange(TT)
    for b in range(B):
        oh = np.zeros((128, 2, TT), np.float32)
        xb = x_i[b]
        oh[xb % 128, xb // 128, jj] = 1.0
        in_maps.append(
            {
                "oh": np.ascontiguousarray(oh.reshape(128, 2 * TT)).astype(bf16),
                "m": np.ascontiguousarray(m_bf[b]),
                "embr": embr,
                "w64": w64,
                "mw1": mw1T,
                "bias": biases,
            }
        )
    return in_maps


def _assemble(blocks) -> np.ndarray:
    """[4, 128, 509] device blocks -> [1018, 250] (mel, text)."""
    blocks = np.asarray(blocks).astype(np.float32)
    out2 = np.empty((TTO, TMO), np.float32)
    out2[0:128, 0:509] = blocks[0]
    out2[0:128, 509:TMO] = blocks[1]
    out2[128:TTO, 0:509] = blocks[2][0 : TTO - 128]
    out2[128:TTO, 509:TMO] = blocks[3][0 : TTO - 128]
    return out2.T


def kernel(**inputs) -> np.ndarray:
    nc = _get_nc()
    in_maps = _prep_in_maps(**inputs)
    res = run_bass_kernel_spmd(nc, in_maps, core_ids=list(range(B)))
    out = np.stack([_assemble(np.asarray(res.results[i]["out"])) for i in range(B)])
    return np.ascontiguousarray(out).astype(np.float32)


# revision 47
# speedup vs baseline: 1.4237x; 1.0255x over previous
"""Trainium2 Bass kernel for the Aligner module (text/mel conv stacks + pairwise L2).

Strategy: data-parallel over batch B=8 -> one sample per NeuronCore, zero
cross-core communication. Per core:
  - embed x via one-hot matmul (one-hot built on host as input marshalling)
  - 3x conv1d (K=3, VALID) per branch as shifted matmuls, bf16 in / f32 psum
  - pairwise distance via a single augmented matmul per 128-row tile:
        d2[i,j] = sum_c mm[c,i]*(-2*tt[c,j]) + nmm[i]*1 + 1*ntt[j]
    (rows 0..63 = channels, rows 64/65 = norm / ones)
  - out = -sqrt(d2) via ACT sqrt + DVE negate, DMA per 2-tile pair.

Scheduling notes: input DMAs are spread across engines so they run in
parallel right after the startup barrier; text-branch matmuls are emitted
between mel tiles so the PE has independent work while conv epilogues run;
conv epilogues are split at the 512/514 boundary so the next layer's first
tile only waits on a tiny boundary chunk.
"""

import numpy as np
import ml_dtypes

import concourse.bass as bass
import concourse.mybir as mybir
import concourse.tile as tile
from concourse import bacc
from concourse.bass_utils import run_bass_kernel_spmd

# Problem constants (hardcoded per harness contract)
B = 8
TT = 256          # text tokens
TM = 1024         # mel frames
V = 256           # vocab
C = 64            # channels
MEL = 80          # mel bins
TTO = TT - 6      # 250
TMO = TM - 6      # 1018

F32 = mybir.dt.float32
BF16 = mybir.dt.bfloat16

# bias pack columns
B_TB1, B_TB2, B_TB3, B_MB1, B_MB2, B_MB3 = range(6)
IDENT = mybir.ActivationFunctionType.Identity


def _build_nc():
    nc = bacc.Bacc(num_swdge_queues=2)

    oh_p = nc.declare_dram_parameter("oh", [128, 2 * TT], BF16, isOutput=False)
    m_p = nc.declare_dram_parameter("m", [MEL, TM], BF16, isOutput=False)
    embr_p = nc.declare_dram_parameter("embr", [128, 128], BF16, isOutput=False)
    # w64 pack: cols [0:576) = text layers (l*192 + k*64 + co), [576:768) mw2T,
    # [768:960) mw3T; all [cin, k, cout] with cin on partitions.
    w64_p = nc.declare_dram_parameter("w64", [64, 960], BF16, isOutput=False)
    mw1_p = nc.declare_dram_parameter("mw1", [MEL, 192], BF16, isOutput=False)
    bias_p = nc.declare_dram_parameter("bias", [64, 10], F32, isOutput=False)
    # output as 8 contiguous [<=128 text, 256 mel] blocks (bb = jt*4 + q) in
    # bf16; host reassembles [250, 1018], upcasts, transposes during unshard.
    out_p = nc.declare_dram_parameter("out", [8, 128, 256], BF16, isOutput=True)

    with tile.TileContext(nc) as tc:
        with (
            tc.tile_pool(name="singles", bufs=1) as singles,
            tc.tile_pool(name="dist", bufs=4) as dpool,
            tc.tile_pool(name="psA", bufs=3, space="PSUM") as psA,
            tc.tile_pool(name="psN", bufs=1, space="PSUM") as psN,
            tc.tile_pool(name="psD", bufs=3, space="PSUM") as psD,
        ):
            # ---- input DMAs: one ordered HWDGE queue, critical tensors first ----
            embr_t = singles.tile([128, 128], BF16)
            nc.sync.dma_start(out=embr_t, in_=embr_p[:, :])
            oh_t = singles.tile([128, 2 * TT], BF16)
            nc.sync.dma_start(out=oh_t[:, 0:TT], in_=oh_p[:, 0:TT])
            nc.sync.dma_start(out=oh_t[:, TT : 2 * TT], in_=oh_p[:, TT : 2 * TT])
            w64_t = singles.tile([64, 960], BF16)
            nc.sync.dma_start(out=w64_t, in_=w64_p[:, :])
            m_t = singles.tile([MEL, TM], BF16)
            nc.sync.dma_start(out=m_t[:, 0:514], in_=m_p[:, 0:514])
            nc.sync.dma_start(out=m_t[:, 514:TM], in_=m_p[:, 514:TM])
            bias_t = singles.tile([64, 10], F32)
            nc.gpsimd.dma_start(out=bias_t, in_=bias_p[:, :])
            mw1_t = singles.tile([MEL, 192], BF16)
            nc.gpsimd.dma_start(out=mw1_t, in_=mw1_p[:, :])

            # ---- ACT table preloads (after DMA issue, before first use) ----
            dummy = singles.tile([1, 2], F32)
            nc.vector.memset(dummy, 0.0)
            dummy2 = singles.tile([1, 2], F32)
            nc.scalar.sqrt(dummy2, dummy)
            nc.scalar.activation(dummy2, dummy, IDENT, bias=0.0)
            nc.scalar.copy(dummy2, dummy)

            # [ones, zeros] / [zeros, ones] column pairs for the [2, N] norm matmuls
            ones2 = singles.tile([64, 2], BF16)
            nc.vector.memset(ones2[:, 0:1], 1.0)
            nc.vector.memset(ones2[:, 1:2], 0.0)
            zo2 = singles.tile([64, 2], BF16)
            nc.vector.memset(zo2[:, 0:1], 0.0)
            nc.vector.memset(zo2[:, 1:2], 1.0)

            # ---- tiles ----
            # all 4 distance output blocks live in one tile so a single
            # dma_start ships them (one instruction spreads its packets
            # across all 16 DMA engines; later stores on a queue do not)
            d_all = singles.tile([128, 2048], BF16)
            nc.vector.memset(d_all, 0.0)
            tt_aug = singles.tile([66, TT], BF16)
            mm_aug = singles.tile([66, TMO], BF16)
            mm1 = singles.tile([64, TM - 2], BF16)
            mm2 = singles.tile([64, TM - 4], BF16)
            t0 = singles.tile([64, TT], BF16)
            t1 = singles.tile([64, 254], BF16)
            t2 = singles.tile([64, 252], BF16)

            def conv3x(ps, wt, wofs, src, n0, nsz):
                for k in range(3):
                    nc.tensor.matmul(
                        ps[:, 0:nsz],
                        wt[:, wofs + k * 64 : wofs + (k + 1) * 64],
                        src[:, n0 + k : n0 + k + nsz],
                        start=(k == 0),
                        stop=(k == 2),
                    )

            def epi_act(dst_ap, ps_ap, bcol, scale=1.0):
                nc.scalar.activation(
                    out=dst_ap, in_=ps_ap, func=IDENT,
                    bias=bias_t[:, bcol : bcol + 1], scale=scale,
                )

            def epi_dve(dst_ap, ps_ap, bcol, s2=None):
                nc.vector.tensor_scalar(
                    out=dst_ap, in0=ps_ap,
                    scalar1=bias_t[:, bcol : bcol + 1],
                    scalar2=s2,
                    op0=mybir.AluOpType.add,
                    **({"op1": mybir.AluOpType.mult} if s2 is not None else {}),
                )

            # ---- embed ----
            ps_e_full = psA.tile([64, 512], F32, tag="convps")
            ps_e = ps_e_full[:, 0:TT]
            nc.tensor.matmul(ps_e, embr_t[:, 0:64], oh_t[:, 0:TT], start=True, stop=False)
            nc.tensor.matmul(
                ps_e, embr_t[:, 64:128], oh_t[:, TT : 2 * TT], start=False, stop=True
            )
            nc.scalar.copy(t0, ps_e)

            # ---- interleaved mel/text conv emission ----
            # mel conv1 tile0
            ps_m10 = psA.tile([64, 512], F32, tag="convps")
            conv3x(ps_m10, mw1_t, 0, m_t, 0, 512)
            epi_act(mm1[:, 0:512], ps_m10[:, 0:512], B_MB1)
            # text conv1
            ps_t1 = psA.tile([64, 512], F32, tag="convps")
            conv3x(ps_t1, w64_t, 0, t0, 0, 254)
            epi_dve(t1, ps_t1[:, 0:254], B_TB1)
            # mel conv1 tile1 (epilogue split: tiny boundary chunk first)
            ps_m11 = psA.tile([64, 512], F32, tag="convps")
            conv3x(ps_m11, mw1_t, 0, m_t, 512, 510)
            nc.vector.tensor_scalar(
                out=mm1[:, 512:514], in0=ps_m11[:, 0:2],
                scalar1=bias_t[:, B_MB1 : B_MB1 + 1], scalar2=None,
                op0=mybir.AluOpType.add,
            )
            epi_dve(mm1[:, 514:1022], ps_m11[:, 2:510], B_MB1)
            # text conv2
            ps_t2 = psA.tile([64, 512], F32, tag="convps")
            conv3x(ps_t2, w64_t, 192, t1, 0, 252)
            epi_dve(t2, ps_t2[:, 0:252], B_TB2)
            # mel conv2 tile0
            ps_m20 = psA.tile([64, 512], F32, tag="convps")
            conv3x(ps_m20, w64_t, 576, mm1, 0, 512)
            epi_act(mm2[:, 0:512], ps_m20[:, 0:512], B_MB2)
            # text conv3 -> tt_aug rows 0..63 = conv3 + tb3
            ps_t3 = psA.tile([64, 512], F32, tag="convps")
            conv3x(ps_t3, w64_t, 384, t2, 0, 250)
            epi_dve(tt_aug[0:64, 0:TTO], ps_t3[:, 0:TTO], B_TB3)
            # mel conv2 tile1
            ps_m21 = psA.tile([64, 512], F32, tag="convps")
            conv3x(ps_m21, w64_t, 576, mm1, 512, 508)
            nc.vector.tensor_scalar(
                out=mm2[:, 512:514], in0=ps_m21[:, 0:2],
                scalar1=bias_t[:, B_MB2 : B_MB2 + 1], scalar2=None,
                op0=mybir.AluOpType.add,
            )
            epi_dve(mm2[:, 514:1020], ps_m21[:, 2:508], B_MB2)

            # ---- text norm chain: tt_aug rows 64,65 = (ntt, 1.0) ----
            sq_tt = singles.tile([64, TTO], BF16)
            nc.vector.tensor_mul(sq_tt, tt_aug[0:64, 0:TTO], tt_aug[0:64, 0:TTO])
            ps_ntt = psN.tile([2, 1024], F32, tag="norm")
            nc.tensor.matmul(ps_ntt[:, 0:TTO], ones2, sq_tt, start=True, stop=True)
            # row0: ntt + 0 ; row1: 0 + 1
            nc.scalar.activation(
                out=tt_aug[64:66, 0:TTO], in_=ps_ntt[:, 0:TTO], func=IDENT,
                bias=bias_t[0:2, 7:8], scale=1.0,
            )

            # ---- mel conv3 + norms, split in halves at column 512 ----
            sq_mm = singles.tile([64, TMO], BF16)
            ps_nmm = psN.tile([2, 1024], F32, tag="norm")

            # mel conv3 psum tiles
            ps_m30 = psA.tile([64, 512], F32, tag="convps")
            conv3x(ps_m30, w64_t, 768, mm2, 0, 512)
            ps_m31 = psA.tile([64, 512], F32, tag="convps")
            conv3x(ps_m31, w64_t, 768, mm2, 512, 506)

            # conv3 epilogue + norm + distance, pipelined per 256-col quarter
            QS = [0, 256, 512, 768]
            for q in range(4):
                qs = QS[q]
                w = min(256, TMO - qs)
                pst = ps_m30 if q < 2 else ps_m31
                lo = qs - (0 if q < 2 else 512)
                # rows 0..63 = -2 * (conv3 + mb3)
                if q % 2 == 0:
                    # ACT computes in*scale + bias, so bias col 8 holds -2*mb3
                    epi_act(mm_aug[0:64, qs : qs + w], pst[:, lo : lo + w], 8, scale=-2.0)
                else:
                    epi_dve(mm_aug[0:64, qs : qs + w], pst[:, lo : lo + w], B_MB3, s2=-2.0)
                nc.vector.tensor_mul(
                    sq_mm[:, qs : qs + w], mm_aug[0:64, qs : qs + w], mm_aug[0:64, qs : qs + w]
                )
                nc.tensor.matmul(
                    ps_nmm[:, qs : qs + w], zo2, sq_mm[:, qs : qs + w], start=True, stop=True
                )
                # row0: 0*0.25 + 1 ; row1: 4*nmm*0.25 + 0
                nc.scalar.activation(
                    out=mm_aug[64:66, qs : qs + w], in_=ps_nmm[:, qs : qs + w],
                    func=IDENT, bias=bias_t[0:2, 6:7], scale=0.25,
                )
                # distance for this mel quarter, both text tiles
                for jt in range(2):
                    j0 = jt * 128
                    cnt = min(128, TTO - j0)  # 128, 122
                    psd = psD.tile([128, 256], F32, tag="psd")
                    nc.tensor.matmul(
                        psd[0:cnt, 0:w],
                        tt_aug[:, j0 : j0 + cnt],
                        mm_aug[:, qs : qs + w],
                        start=True,
                        stop=True,
                    )
                    d_s = dpool.tile([128, 256], F32, tag="d_s")
                    nc.scalar.sqrt(d_s[0:cnt, 0:w], psd[0:cnt, 0:w])
                    bb = jt * 4 + q
                    nc.vector.tensor_scalar_mul(
                        d_all[0:cnt, bb * 256 : bb * 256 + w], d_s[0:cnt, 0:w], -1.0
                    )

            # single store instruction for the whole [8, 128, 256] output
            src = bass.AP(
                tensor=d_all.tensor,
                offset=d_all.offset,
                ap=[d_all.ap[0], [256, 8], [1, 256]],
            )
            base = out_p[:, :, :]
            dst = bass.AP(
                tensor=base.tensor,
                offset=base.offset,
                ap=[[256, 128], [128 * 256, 8], [1, 256]],
            )
            nc.scalar.dma_start(out=dst, in_=src)

    nc.finalize()
    return nc


_NC_CACHE = {}


def _get_nc():
    if "nc" not in _NC_CACHE:
        _NC_CACHE["nc"] = _build_nc()
    return _NC_CACHE["nc"]


def _prep_in_maps(x, m, emb, tw1, tb1, tw2, tb2, tw3, tb3, mw1, mb1, mw2, mb2, mw3, mb3):
    bf16 = ml_dtypes.bfloat16

    # emb [256, 64] -> [128, 2*64]: embr[p, h*64+c] = emb[h*128+p, c]
    embr = (
        np.ascontiguousarray(
            emb.astype(np.float32).reshape(2, 128, C).transpose(1, 0, 2).reshape(128, 128)
        ).astype(bf16)
    )

    def wT(w):  # [cout, cin, 3] -> [cin, 3*cout] with (k, cout) packing
        return np.ascontiguousarray(
            w.astype(np.float32).transpose(1, 2, 0).reshape(w.shape[1], 3 * w.shape[0])
        )

    w64 = np.concatenate(
        [wT(tw1), wT(tw2), wT(tw3), wT(mw2), wT(mw3)], axis=1
    ).astype(bf16)  # [64, 960]
    mw1T = wT(mw1).astype(bf16)  # [80, 192]

    biases = np.zeros((64, 10), np.float32)
    for col, b in zip(range(6), (tb1, tb2, tb3, mb1, mb2, mb3)):
        biases[:, col] = b.astype(np.float32)
    # col 6: bias for mm_aug rows 64,65 -> (0+1, nmm+0); col 7: tt_aug rows -> (ntt+0, 0+1)
    biases[0, 6] = 1.0
    biases[1, 6] = 0.0
    biases[0, 7] = 0.0
    biases[1, 7] = 1.0
    # col 8: -2*mb3 for the ACT variant of the mel conv3 epilogue
    biases[:, 8] = -2.0 * mb3.astype(np.float32)

    # one-hot encoding of x: oh[v%128, (v//128)*256 + j] = (x[j] == v)
    x_i = np.asarray(x).astype(np.int64)
    m_bf = np.asarray(m).astype(np.float32).astype(bf16)

    in_maps = []
    jj = np.arange(TT)
    for b in range(B):
        oh = np.zeros((128, 2, TT), np.float32)
        xb = x_i[b]
        oh[xb % 128, xb // 128, jj] = 1.0
        in_maps.append(
            {
                "oh": np.ascontiguousarray(oh.reshape(128, 2 * TT)).astype(bf16),
                "m": np.ascontiguousarray(m_bf[b]),
                "embr": embr,
                "w64": w64,
                "mw1": mw1T,
                "bias": biases,
            }
        )
    return in_maps


def _assemble(blocks) -> np.ndarray:
    """[8, 128, 256] device blocks (bb = jt*4 + q) -> [1018, 250] (mel, text)."""
    blocks = np.asarray(blocks).astype(np.float32)
    out2 = np.empty((TTO, TMO), np.float32)
    for jt in range(2):
        j0 = jt * 128
        cnt = min(128, TTO - j0)
        for q in range(4):
            qs = q * 256
            w = min(256, TMO - qs)
            out2[j0 : j0 + cnt, qs : qs + w] = blocks[jt * 4 + q][0:cnt, 0:w]
    return out2.T


def kernel(**inputs) -> np.ndarray:
    nc = _get_nc()
    in_maps = _prep_in_maps(**inputs)
    res = run_bass_kernel_spmd(nc, in_maps, core_ids=list(range(B)))
    out = np.stack([_assemble(np.asarray(res.results[i]["out"])) for i in range(B)])
    return np.ascontiguousarray(out).astype(np.float32)
